# revision 8
# baseline (speedup 1.0000x reference)
"""Trainium2 Bass kernel for nn_BasicBlock (dense_cnn, active-shift block).

Data-parallel over batch: 32 images -> 4 per NeuronCore across 8 cores.
Per-core layout: channels on SBUF partitions, pixels (H*W) on the free dim.

Math restructure (validated vs the jax reference in fp32 to ~1e-7):
  - bn1+relu:  relu(s1*z + t1) = s1 * relu(z + t1/s1); the s1 scale is folded
    into the columns of w1, so bn1 is a single add+max tensor_scalar on
    VectorE (bf16, 4x mode).
  - conv1 (groups=2, bf16): two matmuls per pixel tile.  PE matmul outputs
    must start at partition 0 or 64, so the 96 fmap channels live interleaved
    on partitions [0:48] and [64:112]; partitions [48:64] are written zero via
    zero weight columns.  Everything after conv1 uses this padded
    112-partition layout (elementwise ops cost by free dim only, so the dead
    partitions are free); the fmap DMA and conv2 weights fold it back.
  - bn2+relu folded into the row pass: b = relu(fmap + t2) (one tensor_scalar),
    then the separable-bilinear row pass is v = wr1*b (tensor_scalar) plus two
    affine_then_add custom-DVE ops (v[r] += wr0*b[r-1], v[r] += wr2*b[r+1]);
    the bn2 scale s2 is folded into wr.
  - conv2 (col taps folded into weights) is a block-diagonal matmul over the
    padded layout.  The +x residual: channels 48:95 accumulate in PSUM via a
    K=48 shifted-identity matmul from g1_raw's x half; channels 0:47 either
    the same way from g0_raw (PE chunks) or fused into the PSUM->SBUF
    eviction as a VectorE tensor_tensor add (Vector chunks) -- split chosen
    to balance PE vs Vector occupancy.  No separate xres copy (saves ~5 MB of
    DMA-engine traffic per core).

dtype strategy: inputs are cast f32->bf16 by the load DMAs (GpSimd-initiated
casting DMAs); loads are issued at half-image granularity so image 0's bn1
can start as soon as possible.  Outputs are produced as bf16, DMA'd as bf16
and widened to f32 on the host.  End-to-end absmax-relative error ~6e-3.

Spatial tiling: 7 rows (392 px) per PSUM bank; pairs of banks share one PSUM
tile so evictions run at 784-px granularity.
"""

import os
import numpy as np
import ml_dtypes

import concourse.bass as bass
import concourse.bacc as bacc
import concourse.mybir as mybir
from concourse import tile
from concourse.bass_utils import run_bass_kernel_spmd

EPS = 1e-5
N_CORES = 8
N_PER = 4            # images per core
C = 96
CP = 112             # padded channel count for the post-conv1 layout
H = 56
W = 56
PIX = H * W          # 3136
HALF = PIX // 2      # 1568 (28 rows)
RT = 7               # rows per spatial tile
TW = RT * W          # 392 pixels per tile (one PSUM bank each)
NT = H // RT         # 8 tiles per image
NPAIR = NT // 2      # 4 two-bank chunks per image
BANK = 512           # fp32 elems per PSUM bank

# pair-chunks whose ch0:48 residual rides the PE (others ride VectorE)
X0_PE_CHUNKS = (0, 2)

f32 = mybir.dt.float32
bf16 = mybir.dt.bfloat16

LAST_EXEC_NS = None


def _build_nc():
    nc = bacc.Bacc("TRN2", target_bir_lowering=False, debug=False, num_swdge_queues=4)

    x_ext = nc.declare_dram_parameter("x", [N_PER, C, PIX], f32, isOutput=False)
    p_ext = nc.declare_dram_parameter("prev", [N_PER, C, PIX], f32, isOutput=False)
    bias1_ext = nc.declare_dram_parameter("bias1", [C, 2], f32, isOutput=False)
    t2_ext = nc.declare_dram_parameter("t2", [CP, 1], f32, isOutput=False)
    w1t_ext = nc.declare_dram_parameter("w1t", [C, CP], bf16, isOutput=False)
    w2x_ext = nc.declare_dram_parameter("w2x", [CP, 336], bf16, isOutput=False)
    wr_ext = nc.declare_dram_parameter("wr", [CP, 3], f32, isOutput=False)
    resx_ext = nc.declare_dram_parameter("resx", [C, CP], bf16, isOutput=False)
    out_ext = nc.declare_dram_parameter("out", [N_PER, C, PIX], bf16, isOutput=True)
    fmap_ext = nc.declare_dram_parameter("fmap", [N_PER, C, PIX], bf16, isOutput=True)

    with tile.TileContext(nc) as tc:
        with (
            tc.tile_pool(name="consts", bufs=1) as cpool,
            tc.tile_pool(name="raw", bufs=2) as rawp,
            tc.tile_pool(name="act", bufs=2) as actp,
            tc.tile_pool(name="bv", bufs=2) as bvp,
            tc.tile_pool(name="outs", bufs=2) as outp,
            tc.tile_pool(name="fpsum", bufs=2, space="PSUM") as fpsum,
            tc.tile_pool(name="opsum", bufs=2, space="PSUM") as opsum,
        ):
            w1_sb = cpool.tile([C, CP], bf16)
            nc.sync.dma_start(out=w1_sb[:], in_=w1t_ext[:])
            w2_sb = cpool.tile([CP, 336], bf16)
            nc.sync.dma_start(out=w2_sb[:], in_=w2x_ext[:])
            wr_sb = cpool.tile([CP, 3], f32)
            nc.sync.dma_start(out=wr_sb[:], in_=wr_ext[:])
            bias1_sb = cpool.tile([C, 2], f32)
            nc.sync.dma_start(out=bias1_sb[:], in_=bias1_ext[:])
            t2_sb = cpool.tile([CP, 1], f32)
            nc.sync.dma_start(out=t2_sb[:], in_=t2_ext[:])
            resx0_sb = cpool.tile([48, CP], bf16)
            nc.sync.dma_start(out=resx0_sb[:], in_=resx_ext[0:48, :])
            resx1_sb = cpool.tile([48, CP], bf16)
            nc.sync.dma_start(out=resx1_sb[:], in_=resx_ext[48:96, :])

            def emit_dma_loads(n):
                # group0 input = concat channels 0..95  = [x[0:48], prev[48:96]]
                # group1 input = concat channels 96..191 = [x[48:96], prev[0:48]]
                # casting DMAs (f32 -> bf16 in flight), half-image granularity
                g0_raw = rawp.tile([C, PIX], bf16, tag="g0raw", name=f"g0_raw{n}")
                g1_raw = rawp.tile([C, PIX], bf16, tag="g1raw", name=f"g1_raw{n}")
                for hs in (slice(0, HALF), slice(HALF, PIX)):
                    nc.gpsimd.dma_start(out=g0_raw[0:48, hs], in_=x_ext[n, 0:48, hs])
                    nc.gpsimd.dma_start(out=g0_raw[48:96, hs], in_=p_ext[n, 48:96, hs])
                    nc.gpsimd.dma_start(out=g1_raw[0:48, hs], in_=x_ext[n, 48:96, hs])
                    nc.gpsimd.dma_start(out=g1_raw[48:96, hs], in_=p_ext[n, 0:48, hs])
                return g0_raw, g1_raw

            def emit_bn1(n, g0_raw, g1_raw):
                # bn1 + relu (scale folded into w1): a = max(z + bias1, 0)
                g0_act = actp.tile([C, PIX], bf16, tag="g0act", name=f"g0_act{n}")
                g1_act = actp.tile([C, PIX], bf16, tag="g1act", name=f"g1_act{n}")
                for hs in (slice(0, HALF), slice(HALF, PIX)):
                    nc.vector.tensor_scalar(
                        g0_act[:, hs], g0_raw[:, hs], bias1_sb[:, 0:1], 0.0,
                        mybir.AluOpType.add, mybir.AluOpType.max,
                    )
                    nc.vector.tensor_scalar(
                        g1_act[:, hs], g1_raw[:, hs], bias1_sb[:, 1:2], 0.0,
                        mybir.AluOpType.add, mybir.AluOpType.max,
                    )
                return g0_act, g1_act

            raws = [None] * N_PER
            acts = [None] * N_PER
            raws[0] = emit_dma_loads(0)
            acts[0] = emit_bn1(0, *raws[0])
            raws[1] = emit_dma_loads(1)

            for n in range(N_PER):
                g0_raw, g1_raw = raws[n]
                g0_act, g1_act = acts[n]

                b_sb = bvp.tile([CP, PIX], bf16, tag="b")
                v_sb = bvp.tile([CP, PIX], bf16, tag="v")
                fmap_sb = outp.tile([CP, PIX], bf16, tag="fmap")
                out_sb = outp.tile([CP, PIX], bf16, tag="out")

                # conv1 (groups=2) + fmap eviction, per 2-bank chunk
                for cth in range(NPAIR):
                    fp = fpsum.tile([CP, 2 * BANK], f32, tag="fp")
                    for k in range(2):
                        t = 2 * cth + k
                        sl = slice(t * TW, (t + 1) * TW)
                        pb = slice(k * BANK, k * BANK + TW)
                        nc.tensor.matmul(
                            fp[0:64, pb], w1_sb[:, 0:64],
                            g0_act[:, sl], start=True, stop=True,
                        )
                        nc.tensor.matmul(
                            fp[64:112, pb], w1_sb[:, 64:112],
                            g1_act[:, sl], start=True, stop=True,
                        )
                    fpv = fp.rearrange("p (b w) -> p b w", w=BANK)[:, :, 0:TW]
                    csl = slice(cth * 2 * TW, (cth + 1) * 2 * TW)
                    fv = fmap_sb[:, csl].rearrange("p (b w) -> p b w", w=TW)
                    nc.scalar.activation(
                        fv, fpv, mybir.ActivationFunctionType.Copy,
                    )
                    if cth % 2 == 1:
                        hsl = slice((cth - 1) * 2 * TW, (cth + 1) * 2 * TW)
                        nc.sync.dma_start(out=fmap_ext[n, 0:48, hsl],
                                          in_=fmap_sb[0:48, hsl])
                        nc.sync.dma_start(out=fmap_ext[n, 48:96, hsl],
                                          in_=fmap_sb[64:112, hsl])

                # row pass of the shift (bn2 folded in, s2>0 so relu commutes):
                #   b = relu(fmap + t2);  v = wr1*b
                #   v[r] += wr0*b[r-1];  v[r] += wr2*b[r+1]
                # halves ordered so every read refers to already-written data.
                for h0, h1 in ((0, HALF), (HALF, PIX)):
                    hs = slice(h0, h1)
                    nc.vector.tensor_scalar(
                        b_sb[:, hs], fmap_sb[:, hs], t2_sb[:, 0:1], 0.0,
                        mybir.AluOpType.add, mybir.AluOpType.max,
                    )
                    nc.vector.tensor_scalar(
                        v_sb[:, hs], b_sb[:, hs], wr_sb[:, 1:2], None,
                        mybir.AluOpType.mult,
                    )
                    if h0 == 0:
                        # rows 1..27 += wr0*b[0..26]; rows 0..26 += wr2*b[1..27]
                        nc.vector.affine_then_add(
                            v_sb[:, W:HALF], b_sb[:, 0:HALF - W], v_sb[:, W:HALF],
                            wr_sb[:, 0:1], 0.0,
                        )
                        nc.vector.affine_then_add(
                            v_sb[:, 0:HALF - W], b_sb[:, W:HALF], v_sb[:, 0:HALF - W],
                            wr_sb[:, 2:3], 0.0,
                        )
                    else:
                        # rows 28..55 += wr0*b[27..54]; rows 27..54 += wr2*b[28..55]
                        nc.vector.affine_then_add(
                            v_sb[:, HALF:PIX], b_sb[:, HALF - W:PIX - W],
                            v_sb[:, HALF:PIX], wr_sb[:, 0:1], 0.0,
                        )
                        nc.vector.affine_then_add(
                            v_sb[:, HALF - W:PIX - W], b_sb[:, HALF:PIX],
                            v_sb[:, HALF - W:PIX - W], wr_sb[:, 2:3], 0.0,
                        )

                # enqueue next image's bn1 on VectorE behind this row pass
                if n + 1 < N_PER:
                    acts[n + 1] = emit_bn1(n + 1, *raws[n + 1])

                v3 = v_sb.rearrange("p (r w) -> p r w", w=W)

                # conv2 (col taps folded into weights) + residual, then evict
                for cth in range(NPAIR):
                    x0_pe = cth in X0_PE_CHUNKS
                    op = opsum.tile([CP, 2 * BANK], f32, tag="op")
                    for k in range(2):
                        t = 2 * cth + k
                        sl = slice(t * TW, (t + 1) * TW)
                        pb = slice(k * BANK, k * BANK + TW)
                        r0 = t * RT
                        op3 = op[:, pb].rearrange("p (r w) -> p r w", w=W)
                        nc.tensor.matmul(
                            op[:, pb], w2_sb[:, 112:224], v_sb[:, sl],
                            start=True, stop=False, skip_group_check=True,
                        )
                        nc.tensor.matmul(
                            op3[:, :, 1:W], w2_sb[:, 0:112],
                            v3[:, r0:r0 + RT, 0:W - 1],
                            start=False, stop=False, skip_group_check=True,
                        )
                        nc.tensor.matmul(
                            op3[:, :, 0:W - 1], w2_sb[:, 224:336],
                            v3[:, r0:r0 + RT, 1:W],
                            start=False, stop=False, skip_group_check=True,
                        )
                        # residual ch 48:95 from g1_raw's x half (K=48)
                        nc.tensor.matmul(
                            op[:, pb], resx1_sb[:], g1_raw[0:48, sl],
                            start=False, stop=not x0_pe, skip_group_check=True,
                        )
                        if x0_pe:
                            # residual ch 0:47 also on the PE
                            nc.tensor.matmul(
                                op[:, pb], resx0_sb[:], g0_raw[0:48, sl],
                                start=False, stop=True, skip_group_check=True,
                            )
                    opv = op.rearrange("p (b w) -> p b w", w=BANK)[:, :, 0:TW]
                    csl = slice(cth * 2 * TW, (cth + 1) * 2 * TW)
                    ov = out_sb[:, csl].rearrange("p (b w) -> p b w", w=TW)
                    if x0_pe:
                        nc.scalar.activation(
                            ov, opv, mybir.ActivationFunctionType.Copy,
                        )
                    else:
                        # ch 48:95 (p64:112) evicted on ScalarE; ch 0:47
                        # (p0:48) on VectorE fused with the +x residual add
                        nc.scalar.activation(
                            ov[64:112], opv[64:112],
                            mybir.ActivationFunctionType.Copy,
                        )
                        gv = g0_raw[0:48, csl].rearrange("p (b w) -> p b w", w=TW)
                        nc.vector.tensor_tensor(
                            ov[0:48], opv[0:48], gv, mybir.AluOpType.add,
                        )
                    if cth % 2 == 1:
                        hsl = slice((cth - 1) * 2 * TW, (cth + 1) * 2 * TW)
                        nc.sync.dma_start(out=out_ext[n, 0:48, hsl],
                                          in_=out_sb[0:48, hsl])
                        nc.sync.dma_start(out=out_ext[n, 48:96, hsl],
                                          in_=out_sb[64:112, hsl])

                # prefetch image n+2's loads (reuses image n's raw buffers,
                # so this must be emitted after conv2's residual reads)
                if n + 2 < N_PER:
                    raws[n + 2] = emit_dma_loads(n + 2)

    nc.compile()
    return nc


def _prep_consts(bn1_gamma, bn1_beta, bn1_mean, bn1_var,
                 bn2_gamma, bn2_beta, bn2_mean, bn2_var, w1, w2, shift):
    s1 = bn1_gamma / np.sqrt(bn1_var + EPS)
    t1 = bn1_beta - bn1_mean * s1
    bias1 = (t1 / s1).astype(np.float32).reshape(2, C).T.copy()  # [96, 2]

    # padded index for original fmap channel c
    pidx = np.concatenate([np.arange(48), 64 + np.arange(48)])  # [96]

    s2f = bn2_gamma / np.sqrt(bn2_var + EPS)
    b2f = bn2_beta - bn2_mean * s2f
    t2 = np.zeros((CP, 1), np.float32)
    t2[pidx, 0] = b2f / s2f

    w1m = w1[:, :, 0, 0]  # (96 out, 96 in-per-group)
    w1t = np.zeros((C, CP), np.float32)
    w1t[:, 0:48] = (w1m[0:48] * s1[None, 0:96]).T       # group0 lhsT [96K, 48M]
    w1t[:, 64:112] = (w1m[48:96] * s1[None, 96:192]).T  # group1 lhsT

    dy, dx = shift[:, 0].astype(np.float64), shift[:, 1].astype(np.float64)
    ay = np.floor(dy)
    ax = np.floor(dx)
    fy = dy - ay
    fx = dx - ax
    wrf = np.zeros((C, 3), np.float32)
    wcf = np.zeros((C, 3), np.float32)
    for c in range(C):
        iy = int(ay[c]) + 1   # -1 -> 0, 0 -> 1
        ix = int(ax[c]) + 1
        wrf[c, iy] += 1.0 - fy[c]
        wrf[c, iy + 1] += fy[c]
        wcf[c, ix] += 1.0 - fx[c]
        wcf[c, ix + 1] += fx[c]
    wr = np.zeros((CP, 3), np.float32)
    wr[pidx] = wrf * s2f[:, None]

    w2m = w2[:, :, 0, 0]  # (96 out, 32 in-per-group)
    w2full = np.zeros((C, C), np.float32)
    for g in range(3):
        w2full[32 * g:32 * g + 32, 32 * g:32 * g + 32] = w2m[32 * g:32 * g + 32]
    # conv2 output is ALSO padded: out channel o lands on partition pidx[o]
    w2x = np.zeros((CP, 336), np.float32)
    for k in range(3):
        # lhsT[pidx[c], 112*k + pidx[o]] = w2full[o, c] * wc[c, k]
        w2x[np.ix_(pidx, 112 * k + pidx)] = w2full.T * wcf[:, k:k + 1]

    # residual identity matmuls from the raw x halves (padded-M layout):
    #   rows 0:48  -> lhsT [48,112]: k -> out partition k       (g0_raw[0:48])
    #   rows 48:96 -> lhsT [48,112]: k -> out partition 64+k    (g1_raw[0:48])
    resx = np.zeros((C, CP), np.float32)
    resx[np.arange(48), np.arange(48)] = 1.0
    resx[48 + np.arange(48), 64 + np.arange(48)] = 1.0

    return {
        "bias1": bias1,
        "t2": t2,
        "w1t": w1t.astype(ml_dtypes.bfloat16),
        "w2x": w2x.astype(ml_dtypes.bfloat16),
        "wr": wr,
        "resx": resx.astype(ml_dtypes.bfloat16),
    }


_NC_CACHE = {}


def kernel(x, prev_fmap, bn1_gamma, bn1_beta, bn1_mean, bn1_var,
           bn2_gamma, bn2_beta, bn2_mean, bn2_var, w1, w2, shift):
    global LAST_EXEC_NS
    x = np.ascontiguousarray(np.asarray(x, np.float32))
    prev_fmap = np.ascontiguousarray(np.asarray(prev_fmap, np.float32))
    consts = _prep_consts(
        np.asarray(bn1_gamma, np.float32), np.asarray(bn1_beta, np.float32),
        np.asarray(bn1_mean, np.float32), np.asarray(bn1_var, np.float32),
        np.asarray(bn2_gamma, np.float32), np.asarray(bn2_beta, np.float32),
        np.asarray(bn2_mean, np.float32), np.asarray(bn2_var, np.float32),
        np.asarray(w1, np.float32), np.asarray(w2, np.float32),
        np.asarray(shift, np.float32))

    if "nc" not in _NC_CACHE:
        _NC_CACHE["nc"] = _build_nc()
    nc = _NC_CACHE["nc"]

    NB = x.shape[0]
    xs = x.reshape(N_CORES, N_PER, C, PIX)
    ps = prev_fmap.reshape(N_CORES, N_PER, C, PIX)
    in_maps = [
        {"x": xs[i], "prev": ps[i], **consts}
        for i in range(N_CORES)
    ]

    trace = bool(os.environ.get("CC_KERNEL_TRACE"))
    res = run_bass_kernel_spmd(
        nc, in_maps, core_ids=list(range(N_CORES)), trace=trace,
    )
    LAST_EXEC_NS = res.exec_time_ns

    out = np.empty((NB, C, PIX), np.float32)
    fmap = np.empty((NB, C, PIX), np.float32)
    for i in range(N_CORES):
        out[i * N_PER:(i + 1) * N_PER] = res.results[i]["out"].astype(np.float32)
        fmap[i * N_PER:(i + 1) * N_PER] = res.results[i]["fmap"].astype(np.float32)
    return (out.reshape(NB, C, H, W), fmap.reshape(NB, C, H, W))


# revision 9
# speedup vs baseline: 1.0395x; 1.0395x over previous
"""Trainium2 Bass kernel for nn_BasicBlock (dense_cnn, active-shift block).

Data-parallel over batch: 32 images -> 4 per NeuronCore across 8 cores.
Per-core layout: channels on SBUF partitions, pixels (H*W) on the free dim.

Math restructure (validated vs the jax reference in fp32 to ~1e-7):
  - bn1+relu:  relu(s1*z + t1) = s1 * relu(z + t1/s1); the s1 scale is folded
    into the columns of w1, so bn1 is a single add+max tensor_scalar on
    VectorE (bf16, 4x mode).
  - conv1 (groups=2, bf16): two matmuls per pixel tile.  PE matmul outputs
    must start at partition 0 or 64, so the 96 fmap channels live interleaved
    on partitions [0:48] and [64:112]; partitions [48:64] are written zero via
    zero weight columns.  Everything after conv1 uses this padded
    112-partition layout (elementwise ops cost by free dim only, so the dead
    partitions are free); the fmap DMA and conv2 weights fold it back.
  - bn2+relu folded into the row pass: b = relu(fmap + t2) (one tensor_scalar),
    then the separable-bilinear row pass is v = wr1*b (tensor_scalar) plus two
    affine_then_add custom-DVE ops (v[r] += wr0*b[r-1], v[r] += wr2*b[r+1]);
    the bn2 scale s2 is folded into wr.
  - conv2 (col taps folded into weights) is a block-diagonal matmul over the
    padded layout.  The +x residual: channels 48:95 accumulate in PSUM via a
    K=48 shifted-identity matmul from g1_raw's x half; channels 0:47 either
    the same way from g0_raw (PE chunks) or fused into the PSUM->SBUF
    eviction as a VectorE tensor_tensor add (Vector chunks) -- split chosen
    to balance PE vs Vector occupancy.  No separate xres copy (saves ~5 MB of
    DMA-engine traffic per core).

dtype strategy: inputs are cast f32->bf16 by the load DMAs (GpSimd-initiated
casting DMAs); loads are issued at half-image granularity so image 0's bn1
can start as soon as possible.  Outputs are produced as bf16, DMA'd as bf16
and widened to f32 on the host.  End-to-end absmax-relative error ~6e-3.

Spatial tiling: 7 rows (392 px) per PSUM bank; pairs of banks share one PSUM
tile so evictions run at 784-px granularity.
"""

import os
import numpy as np
import ml_dtypes

import concourse.bass as bass
import concourse.bacc as bacc
import concourse.mybir as mybir
from concourse import tile
from concourse.bass_utils import run_bass_kernel_spmd

EPS = 1e-5
N_CORES = 8
N_PER = 4            # images per core
C = 96
CP = 112             # padded channel count for the post-conv1 layout
H = 56
W = 56
PIX = H * W          # 3136
HALF = PIX // 2      # 1568 (28 rows)
RT = 7               # rows per spatial tile
TW = RT * W          # 392 pixels per tile (one PSUM bank each)
NT = H // RT         # 8 tiles per image
NPAIR = NT // 2      # 4 two-bank chunks per image
BANK = 512           # fp32 elems per PSUM bank

# pair-chunks whose ch0:48 residual rides the PE (others ride VectorE)
X0_PE_CHUNKS = (0, 1, 2, 3)

f32 = mybir.dt.float32
bf16 = mybir.dt.bfloat16

LAST_EXEC_NS = None


def _build_nc():
    nc = bacc.Bacc("TRN2", target_bir_lowering=False, debug=False, num_swdge_queues=4)

    x_ext = nc.declare_dram_parameter("x", [N_PER, C, PIX], f32, isOutput=False)
    p_ext = nc.declare_dram_parameter("prev", [N_PER, C, PIX], f32, isOutput=False)
    bias1_ext = nc.declare_dram_parameter("bias1", [C, 2], f32, isOutput=False)
    t2_ext = nc.declare_dram_parameter("t2", [CP, 1], f32, isOutput=False)
    w1t_ext = nc.declare_dram_parameter("w1t", [C, CP], bf16, isOutput=False)
    w2x_ext = nc.declare_dram_parameter("w2x", [CP, 336], bf16, isOutput=False)
    wr_ext = nc.declare_dram_parameter("wr", [CP, 3], f32, isOutput=False)
    resx_ext = nc.declare_dram_parameter("resx", [C, CP], bf16, isOutput=False)
    out_ext = nc.declare_dram_parameter("out", [N_PER, C, PIX], bf16, isOutput=True)
    fmap_ext = nc.declare_dram_parameter("fmap", [N_PER, C, PIX], bf16, isOutput=True)

    with tile.TileContext(nc) as tc:
        with (
            tc.tile_pool(name="consts", bufs=1) as cpool,
            tc.tile_pool(name="raw", bufs=3) as rawp,
            tc.tile_pool(name="act", bufs=2) as actp,
            tc.tile_pool(name="bv", bufs=2) as bvp,
            tc.tile_pool(name="outs", bufs=2) as outp,
            tc.tile_pool(name="fpsum", bufs=2, space="PSUM") as fpsum,
            tc.tile_pool(name="opsum", bufs=2, space="PSUM") as opsum,
        ):
            w1_sb = cpool.tile([C, CP], bf16)
            nc.sync.dma_start(out=w1_sb[:], in_=w1t_ext[:])
            w2_sb = cpool.tile([CP, 336], bf16)
            nc.sync.dma_start(out=w2_sb[:], in_=w2x_ext[:])
            wr_sb = cpool.tile([CP, 3], f32)
            nc.sync.dma_start(out=wr_sb[:], in_=wr_ext[:])
            bias1_sb = cpool.tile([C, 2], f32)
            nc.sync.dma_start(out=bias1_sb[:], in_=bias1_ext[:])
            t2_sb = cpool.tile([CP, 1], f32)
            nc.sync.dma_start(out=t2_sb[:], in_=t2_ext[:])
            resx0_sb = cpool.tile([48, CP], bf16)
            nc.sync.dma_start(out=resx0_sb[:], in_=resx_ext[0:48, :])
            resx1_sb = cpool.tile([48, CP], bf16)
            nc.sync.dma_start(out=resx1_sb[:], in_=resx_ext[48:96, :])

            def emit_dma_loads(n):
                # group0 input = concat channels 0..95  = [x[0:48], prev[48:96]]
                # group1 input = concat channels 96..191 = [x[48:96], prev[0:48]]
                # casting DMAs (f32 -> bf16 in flight), half-image granularity
                g0_raw = rawp.tile([C, PIX], bf16, tag="g0raw", name=f"g0_raw{n}")
                g1_raw = rawp.tile([C, PIX], bf16, tag="g1raw", name=f"g1_raw{n}")
                for hs in (slice(0, HALF), slice(HALF, PIX)):
                    nc.gpsimd.dma_start(out=g0_raw[0:48, hs], in_=x_ext[n, 0:48, hs])
                    nc.gpsimd.dma_start(out=g0_raw[48:96, hs], in_=p_ext[n, 48:96, hs])
                    nc.gpsimd.dma_start(out=g1_raw[0:48, hs], in_=x_ext[n, 48:96, hs])
                    nc.gpsimd.dma_start(out=g1_raw[48:96, hs], in_=p_ext[n, 0:48, hs])
                return g0_raw, g1_raw

            def emit_bn1(n, g0_raw, g1_raw):
                # bn1 + relu (scale folded into w1): a = max(z + bias1, 0)
                g0_act = actp.tile([C, PIX], bf16, tag="g0act", name=f"g0_act{n}")
                g1_act = actp.tile([C, PIX], bf16, tag="g1act", name=f"g1_act{n}")
                for hs in (slice(0, HALF), slice(HALF, PIX)):
                    nc.vector.tensor_scalar(
                        g0_act[:, hs], g0_raw[:, hs], bias1_sb[:, 0:1], 0.0,
                        mybir.AluOpType.add, mybir.AluOpType.max,
                    )
                    nc.vector.tensor_scalar(
                        g1_act[:, hs], g1_raw[:, hs], bias1_sb[:, 1:2], 0.0,
                        mybir.AluOpType.add, mybir.AluOpType.max,
                    )
                return g0_act, g1_act

            raws = [None] * N_PER
            acts = [None] * N_PER
            raws[0] = emit_dma_loads(0)
            acts[0] = emit_bn1(0, *raws[0])
            raws[1] = emit_dma_loads(1)

            for n in range(N_PER):
                g0_raw, g1_raw = raws[n]
                g0_act, g1_act = acts[n]

                b_sb = bvp.tile([CP, PIX], bf16, tag="b")
                v_sb = bvp.tile([CP, PIX], bf16, tag="v")
                bm_sb = bvp.tile([CP, PIX], bf16, tag="bm")
                bp_sb = bvp.tile([CP, PIX], bf16, tag="bp")
                fmap_sb = outp.tile([CP, PIX], bf16, tag="fmap")
                out_sb = outp.tile([CP, PIX], bf16, tag="out")

                # conv1 (groups=2) + fmap eviction, per 2-bank chunk
                for cth in range(NPAIR):
                    fp = fpsum.tile([CP, 2 * BANK], f32, tag="fp")
                    for k in range(2):
                        t = 2 * cth + k
                        sl = slice(t * TW, (t + 1) * TW)
                        pb = slice(k * BANK, k * BANK + TW)
                        nc.tensor.matmul(
                            fp[0:64, pb], w1_sb[:, 0:64],
                            g0_act[:, sl], start=True, stop=True,
                        )
                        nc.tensor.matmul(
                            fp[64:112, pb], w1_sb[:, 64:112],
                            g1_act[:, sl], start=True, stop=True,
                        )
                    fpv = fp.rearrange("p (b w) -> p b w", w=BANK)[:, :, 0:TW]
                    csl = slice(cth * 2 * TW, (cth + 1) * 2 * TW)
                    fv = fmap_sb[:, csl].rearrange("p (b w) -> p b w", w=TW)
                    nc.scalar.activation(
                        fv, fpv, mybir.ActivationFunctionType.Copy,
                    )
                    if cth % 2 == 1:
                        hsl = slice((cth - 1) * 2 * TW, (cth + 1) * 2 * TW)
                        nc.sync.dma_start(out=fmap_ext[n, 0:48, hsl],
                                          in_=fmap_sb[0:48, hsl])
                        nc.sync.dma_start(out=fmap_ext[n, 48:96, hsl],
                                          in_=fmap_sb[64:112, hsl])

                # row pass of the shift (bn2 folded in, s2>0 so relu commutes):
                #   b = relu(fmap + t2);  v = wr1*b
                #   v[r] += wr0*b[r-1];  v[r] += wr2*b[r+1]
                # halves ordered so every read refers to already-written data.
                for h0, h1 in ((0, HALF), (HALF, PIX)):
                    hs = slice(h0, h1)
                    nc.vector.tensor_scalar(
                        b_sb[:, hs], fmap_sb[:, hs], t2_sb[:, 0:1], 0.0,
                        mybir.AluOpType.add, mybir.AluOpType.max,
                    )
                    nc.vector.tensor_scalar(
                        v_sb[:, hs], b_sb[:, hs], wr_sb[:, 1:2], None,
                        mybir.AluOpType.mult,
                    )
                    nc.vector.tensor_scalar(
                        bm_sb[:, hs], b_sb[:, hs], wr_sb[:, 0:1], None,
                        mybir.AluOpType.mult,
                    )
                    nc.vector.tensor_scalar(
                        bp_sb[:, hs], b_sb[:, hs], wr_sb[:, 2:3], None,
                        mybir.AluOpType.mult,
                    )
                    if h0 == 0:
                        # rows 1..27: bm rows 0..26 ; rows 0..26: bp rows 1..27
                        nc.vector.tensor_tensor(
                            v_sb[:, W:HALF], bm_sb[:, 0:HALF - W], v_sb[:, W:HALF],
                            mybir.AluOpType.add,
                        )
                        nc.vector.tensor_tensor(
                            v_sb[:, 0:HALF - W], bp_sb[:, W:HALF], v_sb[:, 0:HALF - W],
                            mybir.AluOpType.add,
                        )
                    else:
                        # rows 28..55: bm rows 27..54 ; rows 27..54: bp rows 28..55
                        nc.vector.tensor_tensor(
                            v_sb[:, HALF:PIX], bm_sb[:, HALF - W:PIX - W],
                            v_sb[:, HALF:PIX], mybir.AluOpType.add,
                        )
                        nc.vector.tensor_tensor(
                            v_sb[:, HALF - W:PIX - W], bp_sb[:, HALF:PIX],
                            v_sb[:, HALF - W:PIX - W], mybir.AluOpType.add,
                        )

                # prefetch image n+2's loads (bufs=3: reuses image n-1's
                # raw buffers, whose readers are already retired)
                if n + 2 < N_PER:
                    raws[n + 2] = emit_dma_loads(n + 2)
                # enqueue next image's bn1 on VectorE behind this row pass
                if n + 1 < N_PER:
                    acts[n + 1] = emit_bn1(n + 1, *raws[n + 1])

                v3 = v_sb.rearrange("p (r w) -> p r w", w=W)

                # conv2 (col taps folded into weights) + residual, then evict
                for cth in range(NPAIR):
                    x0_pe = cth in X0_PE_CHUNKS
                    op = opsum.tile([CP, 2 * BANK], f32, tag="op")
                    for k in range(2):
                        t = 2 * cth + k
                        sl = slice(t * TW, (t + 1) * TW)
                        pb = slice(k * BANK, k * BANK + TW)
                        r0 = t * RT
                        op3 = op[:, pb].rearrange("p (r w) -> p r w", w=W)
                        nc.tensor.matmul(
                            op[:, pb], w2_sb[:, 112:224], v_sb[:, sl],
                            start=True, stop=False, skip_group_check=True,
                        )
                        nc.tensor.matmul(
                            op3[:, :, 1:W], w2_sb[:, 0:112],
                            v3[:, r0:r0 + RT, 0:W - 1],
                            start=False, stop=False, skip_group_check=True,
                        )
                        nc.tensor.matmul(
                            op3[:, :, 0:W - 1], w2_sb[:, 224:336],
                            v3[:, r0:r0 + RT, 1:W],
                            start=False, stop=False, skip_group_check=True,
                        )
                        # residual ch 48:95 from g1_raw's x half (K=48)
                        nc.tensor.matmul(
                            op[:, pb], resx1_sb[:], g1_raw[0:48, sl],
                            start=False, stop=not x0_pe, skip_group_check=True,
                        )
                        if x0_pe:
                            # residual ch 0:47 also on the PE
                            nc.tensor.matmul(
                                op[:, pb], resx0_sb[:], g0_raw[0:48, sl],
                                start=False, stop=True, skip_group_check=True,
                            )
                    opv = op.rearrange("p (b w) -> p b w", w=BANK)[:, :, 0:TW]
                    csl = slice(cth * 2 * TW, (cth + 1) * 2 * TW)
                    ov = out_sb[:, csl].rearrange("p (b w) -> p b w", w=TW)
                    if x0_pe:
                        nc.scalar.activation(
                            ov, opv, mybir.ActivationFunctionType.Copy,
                        )
                    else:
                        # ch 48:95 (p64:112) evicted on ScalarE; ch 0:47
                        # (p0:48) on VectorE fused with the +x residual add
                        nc.scalar.activation(
                            ov[64:112], opv[64:112],
                            mybir.ActivationFunctionType.Copy,
                        )
                        gv = g0_raw[0:48, csl].rearrange("p (b w) -> p b w", w=TW)
                        nc.vector.tensor_tensor(
                            ov[0:48], opv[0:48], gv, mybir.AluOpType.add,
                        )
                    if cth % 2 == 1:
                        hsl = slice((cth - 1) * 2 * TW, (cth + 1) * 2 * TW)
                        nc.sync.dma_start(out=out_ext[n, 0:48, hsl],
                                          in_=out_sb[0:48, hsl])
                        nc.sync.dma_start(out=out_ext[n, 48:96, hsl],
                                          in_=out_sb[64:112, hsl])

    nc.compile()
    return nc


def _prep_consts(bn1_gamma, bn1_beta, bn1_mean, bn1_var,
                 bn2_gamma, bn2_beta, bn2_mean, bn2_var, w1, w2, shift):
    s1 = bn1_gamma / np.sqrt(bn1_var + EPS)
    t1 = bn1_beta - bn1_mean * s1
    bias1 = (t1 / s1).astype(np.float32).reshape(2, C).T.copy()  # [96, 2]

    # padded index for original fmap channel c
    pidx = np.concatenate([np.arange(48), 64 + np.arange(48)])  # [96]

    s2f = bn2_gamma / np.sqrt(bn2_var + EPS)
    b2f = bn2_beta - bn2_mean * s2f
    t2 = np.zeros((CP, 1), np.float32)
    t2[pidx, 0] = b2f / s2f

    w1m = w1[:, :, 0, 0]  # (96 out, 96 in-per-group)
    w1t = np.zeros((C, CP), np.float32)
    w1t[:, 0:48] = (w1m[0:48] * s1[None, 0:96]).T       # group0 lhsT [96K, 48M]
    w1t[:, 64:112] = (w1m[48:96] * s1[None, 96:192]).T  # group1 lhsT

    dy, dx = shift[:, 0].astype(np.float64), shift[:, 1].astype(np.float64)
    ay = np.floor(dy)
    ax = np.floor(dx)
    fy = dy - ay
    fx = dx - ax
    wrf = np.zeros((C, 3), np.float32)
    wcf = np.zeros((C, 3), np.float32)
    for c in range(C):
        iy = int(ay[c]) + 1   # -1 -> 0, 0 -> 1
        ix = int(ax[c]) + 1
        wrf[c, iy] += 1.0 - fy[c]
        wrf[c, iy + 1] += fy[c]
        wcf[c, ix] += 1.0 - fx[c]
        wcf[c, ix + 1] += fx[c]
    wr = np.zeros((CP, 3), np.float32)
    wr[pidx] = wrf * s2f[:, None]

    w2m = w2[:, :, 0, 0]  # (96 out, 32 in-per-group)
    w2full = np.zeros((C, C), np.float32)
    for g in range(3):
        w2full[32 * g:32 * g + 32, 32 * g:32 * g + 32] = w2m[32 * g:32 * g + 32]
    # conv2 output is ALSO padded: out channel o lands on partition pidx[o]
    w2x = np.zeros((CP, 336), np.float32)
    for k in range(3):
        # lhsT[pidx[c], 112*k + pidx[o]] = w2full[o, c] * wc[c, k]
        w2x[np.ix_(pidx, 112 * k + pidx)] = w2full.T * wcf[:, k:k + 1]

    # residual identity matmuls from the raw x halves (padded-M layout):
    #   rows 0:48  -> lhsT [48,112]: k -> out partition k       (g0_raw[0:48])
    #   rows 48:96 -> lhsT [48,112]: k -> out partition 64+k    (g1_raw[0:48])
    resx = np.zeros((C, CP), np.float32)
    resx[np.arange(48), np.arange(48)] = 1.0
    resx[48 + np.arange(48), 64 + np.arange(48)] = 1.0

    return {
        "bias1": bias1,
        "t2": t2,
        "w1t": w1t.astype(ml_dtypes.bfloat16),
        "w2x": w2x.astype(ml_dtypes.bfloat16),
        "wr": wr,
        "resx": resx.astype(ml_dtypes.bfloat16),
    }


_NC_CACHE = {}


def kernel(x, prev_fmap, bn1_gamma, bn1_beta, bn1_mean, bn1_var,
           bn2_gamma, bn2_beta, bn2_mean, bn2_var, w1, w2, shift):
    global LAST_EXEC_NS
    x = np.ascontiguousarray(np.asarray(x, np.float32))
    prev_fmap = np.ascontiguousarray(np.asarray(prev_fmap, np.float32))
    consts = _prep_consts(
        np.asarray(bn1_gamma, np.float32), np.asarray(bn1_beta, np.float32),
        np.asarray(bn1_mean, np.float32), np.asarray(bn1_var, np.float32),
        np.asarray(bn2_gamma, np.float32), np.asarray(bn2_beta, np.float32),
        np.asarray(bn2_mean, np.float32), np.asarray(bn2_var, np.float32),
        np.asarray(w1, np.float32), np.asarray(w2, np.float32),
        np.asarray(shift, np.float32))

    if "nc" not in _NC_CACHE:
        _NC_CACHE["nc"] = _build_nc()
    nc = _NC_CACHE["nc"]

    NB = x.shape[0]
    xs = x.reshape(N_CORES, N_PER, C, PIX)
    ps = prev_fmap.reshape(N_CORES, N_PER, C, PIX)
    in_maps = [
        {"x": xs[i], "prev": ps[i], **consts}
        for i in range(N_CORES)
    ]

    trace = bool(os.environ.get("CC_KERNEL_TRACE"))
    res = run_bass_kernel_spmd(
        nc, in_maps, core_ids=list(range(N_CORES)), trace=trace,
    )
    LAST_EXEC_NS = res.exec_time_ns

    out = np.empty((NB, C, PIX), np.float32)
    fmap = np.empty((NB, C, PIX), np.float32)
    for i in range(N_CORES):
        out[i * N_PER:(i + 1) * N_PER] = res.results[i]["out"].astype(np.float32)
        fmap[i * N_PER:(i + 1) * N_PER] = res.results[i]["fmap"].astype(np.float32)
    return (out.reshape(NB, C, H, W), fmap.reshape(NB, C, H, W))


# revision 12
# speedup vs baseline: 1.4313x; 1.3769x over previous
"""Trainium2 Bass kernel for nn_BasicBlock (dense_cnn, active-shift block).

Data-parallel over batch: 32 images -> 4 per NeuronCore across 8 cores.
Per-core layout: channels on SBUF partitions, pixels (H*W) on the free dim.

Math restructure (validated vs the jax reference in fp32 to ~1e-7):
  - bn1+relu:  relu(s1*z + t1) = s1 * relu(z + t1/s1); the s1 scale is folded
    into the columns of w1, so bn1 is a single add+max tensor_scalar on
    VectorE (bf16, 4x mode).
  - conv1 (groups=2, bf16): two matmuls per pixel tile.  PE matmul outputs
    must start at partition 0 or 64, so the 96 fmap channels live interleaved
    on partitions [0:48] and [64:112]; partitions [48:64] are written zero via
    zero weight columns.  Everything after conv1 uses this padded
    112-partition layout (elementwise ops cost by free dim only, so the dead
    partitions are free); the fmap DMA and conv2 weights fold it back.
  - bn2+relu: ScalarE activation (per-partition scale/bias) from PSUM -> bf16.
  - active_shift is separable bilinear: a row pass on VectorE
    (v = wr0*b; bm = wrm*b; bp = wrp*b; v += shift(bm); v += shift(bp) --
    tensor_scalar 4x + tensor_tensor 2x only, no 1x-mode ops) and a column
    pass folded into conv2's weights (3 matmuls with column-shifted APs).
  - conv2 (groups=3) is a block-diagonal matmul over the padded layout; the
    +x residual is accumulated in PSUM via two shifted-identity matmuls from
    the bf16 raw tiles; ScalarE evicts the result.

dtype strategy: inputs are cast f32->bf16 by the load DMAs (GpSimd-initiated
casting DMAs; the GpSimd ALU pipeline stays empty -- its tensor ops are both
slow and poison concurrent VectorE ops via SBUF port sharing).  Outputs are
produced as bf16, DMA'd as bf16 (halves output HBM traffic) and widened to
f32 on the host.  End-to-end absmax-relative error ~3e-3.

Spatial tiling: 7 rows (392 px) per PSUM bank; pairs of banks share one PSUM
tile so bn2 / copies run at 784-px granularity (amortizes per-op overheads).
"""

import os
import numpy as np
import ml_dtypes

import concourse.bass as bass
import concourse.bacc as bacc
import concourse.mybir as mybir
from concourse import tile
from concourse.bass_utils import run_bass_kernel_spmd

EPS = 1e-5
N_CORES = 8
N_PER = 4            # images per core
C = 96
CP = 112             # padded channel count for the post-conv1 layout
H = 56
W = 56
PIX = H * W          # 3136
RT = 7               # rows per spatial tile
TW = RT * W          # 392 pixels per tile (one PSUM bank each)
NT = H // RT         # 8 tiles per image
NPAIR = NT // 2      # 4 two-bank chunks per image
BANK = 512           # fp32 elems per PSUM bank

f32 = mybir.dt.float32
bf16 = mybir.dt.bfloat16
f32r = mybir.dt.float32r

LAST_EXEC_NS = None


def _build_nc():
    nc = bacc.Bacc("TRN2", target_bir_lowering=False, debug=False, num_swdge_queues=4)

    x_ext = nc.declare_dram_parameter("x", [N_PER, C, PIX], f32, isOutput=False)
    p_ext = nc.declare_dram_parameter("prev", [N_PER, C, PIX], f32, isOutput=False)
    bias1_ext = nc.declare_dram_parameter("bias1", [CP, 2], f32, isOutput=False)
    t2_ext = nc.declare_dram_parameter("t2", [CP, 1], f32, isOutput=False)
    w1t_ext = nc.declare_dram_parameter("w1t", [CP, CP], bf16, isOutput=False)
    w2x_ext = nc.declare_dram_parameter("w2x", [CP, 288], bf16, isOutput=False)
    wr_ext = nc.declare_dram_parameter("wr", [CP, 3], f32, isOutput=False)
    resw_ext = nc.declare_dram_parameter("resw", [CP, 96], bf16, isOutput=False)
    zeros_ext = nc.declare_dram_parameter("zeros16", [16, PIX], bf16, isOutput=False)
    out_ext = nc.declare_dram_parameter("out", [N_PER, C, PIX], bf16, isOutput=True)
    fmap_ext = nc.declare_dram_parameter("fmap", [N_PER, C, PIX], bf16, isOutput=True)

    with tile.TileContext(nc) as tc:
        with (
            tc.tile_pool(name="consts", bufs=1) as cpool,
            tc.tile_pool(name="raw", bufs=2) as rawp,
            tc.tile_pool(name="act", bufs=2) as actp,
            tc.tile_pool(name="bv", bufs=2) as bvp,
            tc.tile_pool(name="outs", bufs=2) as outp,
            tc.tile_pool(name="fpsum", bufs=2, space="PSUM") as fpsum,
            tc.tile_pool(name="opsum", bufs=2, space="PSUM") as opsum,
        ):
            w1_sb = cpool.tile([CP, CP], bf16)
            nc.sync.dma_start(out=w1_sb[:], in_=w1t_ext[:])
            w2_sb = cpool.tile([CP, 288], bf16)
            nc.sync.dma_start(out=w2_sb[:], in_=w2x_ext[:])
            wr_sb = cpool.tile([CP, 3], f32)
            nc.sync.dma_start(out=wr_sb[:], in_=wr_ext[:])
            bias1_sb = cpool.tile([CP, 2], f32)
            nc.sync.dma_start(out=bias1_sb[:], in_=bias1_ext[:])
            t2_sb = cpool.tile([CP, 1], f32)
            nc.sync.dma_start(out=t2_sb[:], in_=t2_ext[:])
            resw_sb = cpool.tile([CP, 96], bf16)
            nc.sync.dma_start(out=resw_sb[:], in_=resw_ext[:])

            HALF = PIX // 2

            def emit_loads(n):
                # group0 input = [x[0:48] @ p0:48,    prev[48:96] @ p48:96]
                # group1 input = [prev[0:48] @ p0:48, x[48:96]    @ p48:96]
                # (group1's input-channel order is folded into w1t host-side
                # so both x halves sit partition-aligned with xres)
                # casting DMAs (f32 -> bf16 in flight) via gpsimd rings,
                # half-image granularity for earlier bn1 start
                g0_raw = rawp.tile([C, PIX], bf16, tag="g0raw", name=f"g0_raw{n}")
                g1_raw = rawp.tile([CP, PIX], bf16, tag="g1raw", name=f"g1_raw{n}")
                for hs in (slice(0, HALF), slice(HALF, PIX)):
                    nc.gpsimd.dma_start(out=g0_raw[0:48, hs], in_=x_ext[n, 0:48, hs])
                    nc.gpsimd.dma_start(out=g0_raw[48:96, hs], in_=p_ext[n, 48:96, hs])
                    nc.gpsimd.dma_start(out=g1_raw[64:112, hs], in_=x_ext[n, 48:96, hs])
                    nc.gpsimd.dma_start(out=g1_raw[0:48, hs], in_=p_ext[n, 0:48, hs])
                    # keep the p48:64 hole finite (zero-weight rows in w1/resw)
                    nc.gpsimd.dma_start(out=g1_raw[48:64, hs], in_=zeros_ext[:, hs])

                # bn1 + relu (scale folded into w1): a = max(z + bias1, 0)
                g0_act = actp.tile([C, PIX], bf16, tag="g0act", name=f"g0_act{n}")
                g1_act = actp.tile([CP, PIX], bf16, tag="g1act", name=f"g1_act{n}")
                for hs in (slice(0, HALF), slice(HALF, PIX)):
                    nc.vector.tensor_scalar(
                        g0_act[:, hs], g0_raw[:, hs], bias1_sb[0:96, 0:1], 0.0,
                        mybir.AluOpType.add, mybir.AluOpType.max,
                    )
                    nc.vector.tensor_scalar(
                        g1_act[:, hs], g1_raw[:, hs], bias1_sb[:, 1:2], 0.0,
                        mybir.AluOpType.add, mybir.AluOpType.max,
                    )
                return g0_raw, g1_raw, g0_act, g1_act

            def emit_xres(n, g0_raw, g1_raw):
                # contiguous bf16 copy of x for the single-matmul residual,
                # assembled on VectorE (partition-aligned thanks to the
                # permuted g1 load) instead of a SBUF->SBUF DMA copy
                xres = outp.tile([CP, PIX], bf16, tag="xres", name=f"xres{n}")
                nc.vector.tensor_scalar(
                    xres[0:64, :], g0_raw[0:64, :], 1.0, None,
                    mybir.AluOpType.mult,
                )
                nc.vector.tensor_scalar(
                    xres[64:112, :], g1_raw[64:112, :], 1.0, None,
                    mybir.AluOpType.mult,
                )
                return xres

            nxt = emit_loads(0)
            xres = emit_xres(0, nxt[0], nxt[1])
            for n in range(N_PER):
                g0_raw, g1_raw, g0_act, g1_act = nxt
                if n + 1 < N_PER:
                    nxt = emit_loads(n + 1)

                b_sb = bvp.tile([CP, PIX], bf16, tag="b")
                v_sb = bvp.tile([CP, PIX], bf16, tag="v")
                bm_sb = bvp.tile([CP, PIX], bf16, tag="bm")
                bp_sb = bvp.tile([CP, PIX], bf16, tag="bp")
                fmap_sb = outp.tile([CP, PIX], bf16, tag="fmap")
                out_sb = outp.tile([C, PIX], bf16, tag="out")

                # conv1 (groups=2) + bn2(relu) + fmap eviction, per 2-bank chunk
                for cth in range(NPAIR):
                    fp = fpsum.tile([CP, 2 * BANK], f32, tag="fp")
                    for k in range(2):
                        t = 2 * cth + k
                        sl = slice(t * TW, (t + 1) * TW)
                        pb = slice(k * BANK, k * BANK + TW)
                        nc.tensor.matmul(
                            fp[0:64, pb], w1_sb[0:96, 0:64],
                            g0_act[:, sl], start=True, stop=True,
                        )
                        nc.tensor.matmul(
                            fp[64:112, pb], w1_sb[:, 64:112],
                            g1_act[:, sl], start=True, stop=True,
                        )
                    fpv = fp.rearrange("p (b w) -> p b w", w=BANK)[:, :, 0:TW]
                    csl = slice(cth * 2 * TW, (cth + 1) * 2 * TW)
                    fv = fmap_sb[:, csl].rearrange("p (b w) -> p b w", w=TW)
                    nc.scalar.activation(
                        fv, fpv, mybir.ActivationFunctionType.Copy,
                    )
                    if cth % 2 == 1:
                        hsl = slice((cth - 1) * 2 * TW, (cth + 1) * 2 * TW)
                        nc.sync.dma_start(out=fmap_ext[n, 0:48, hsl],
                                          in_=fmap_sb[0:48, hsl])
                        nc.sync.dma_start(out=fmap_ext[n, 48:96, hsl],
                                          in_=fmap_sb[64:112, hsl])

                # row pass of the shift: v[c,i,:] = sum_oy wr[c,oy]*b[c,i+oy,:]
                # tensor_scalar (4x) + tensor_tensor (2x) only; no 1x STT ops.
                # Two halves, with the cross-half halo rows handled in the
                # second batch so every read refers to already-written data.
                HALF = PIX // 2
                for h0, h1 in ((0, HALF), (HALF, PIX)):
                    hs = slice(h0, h1)
                    # bn2 (scale folded into wr): b' = max(fmap + b2/s2, 0)
                    nc.vector.tensor_scalar(
                        b_sb[:, hs], fmap_sb[:, hs], t2_sb[:, 0:1], 0.0,
                        mybir.AluOpType.add, mybir.AluOpType.max,
                    )
                    nc.vector.tensor_scalar(
                        v_sb[:, hs], b_sb[:, hs], wr_sb[:, 1:2], None,
                        mybir.AluOpType.mult,
                    )
                    nc.vector.tensor_scalar(
                        bm_sb[:, hs], b_sb[:, hs], wr_sb[:, 0:1], None,
                        mybir.AluOpType.mult,
                    )
                    nc.vector.tensor_scalar(
                        bp_sb[:, hs], b_sb[:, hs], wr_sb[:, 2:3], None,
                        mybir.AluOpType.mult,
                    )
                    if h0 == 0:
                        # rows 1..27: bm rows 0..26 ; rows 0..26: bp rows 1..27
                        nc.vector.tensor_tensor(
                            v_sb[:, W:HALF], bm_sb[:, 0:HALF - W], v_sb[:, W:HALF],
                            mybir.AluOpType.add,
                        )
                        nc.vector.tensor_tensor(
                            v_sb[:, 0:HALF - W], bp_sb[:, W:HALF], v_sb[:, 0:HALF - W],
                            mybir.AluOpType.add,
                        )
                    else:
                        # rows 28..55: bm rows 27..54 ; rows 27..54: bp rows 28..55
                        nc.vector.tensor_tensor(
                            v_sb[:, HALF:PIX], bm_sb[:, HALF - W:PIX - W],
                            v_sb[:, HALF:PIX], mybir.AluOpType.add,
                        )
                        nc.vector.tensor_tensor(
                            v_sb[:, HALF - W:PIX - W], bp_sb[:, HALF:PIX],
                            v_sb[:, HALF - W:PIX - W], mybir.AluOpType.add,
                        )

                v3 = v_sb.rearrange("p (r w) -> p r w", w=W)

                # conv2 (col taps folded into weights) + residual, then evict
                for cth in range(NPAIR):
                    op = opsum.tile([C, 2 * BANK], f32, tag="op")
                    for k in range(2):
                        t = 2 * cth + k
                        sl = slice(t * TW, (t + 1) * TW)
                        pb = slice(k * BANK, k * BANK + TW)
                        r0 = t * RT
                        op3 = op[:, pb].rearrange("p (r w) -> p r w", w=W)
                        nc.tensor.matmul(
                            op[:, pb], w2_sb[:, 96:192], v_sb[:, sl],
                            start=True, stop=False, skip_group_check=True,
                        )
                        nc.tensor.matmul(
                            op3[:, :, 1:W], w2_sb[:, 0:96],
                            v3[:, r0:r0 + RT, 0:W - 1],
                            start=False, stop=False, skip_group_check=True,
                        )
                        nc.tensor.matmul(
                            op3[:, :, 0:W - 1], w2_sb[:, 192:288],
                            v3[:, r0:r0 + RT, 1:W],
                            start=False, stop=False, skip_group_check=True,
                        )
                        nc.tensor.matmul(
                            op[:, pb], resw_sb[:], xres[:, sl],
                            start=False, stop=True, skip_group_check=True,
                        )
                    opv = op.rearrange("p (b w) -> p b w", w=BANK)[:, :, 0:TW]
                    csl = slice(cth * 2 * TW, (cth + 1) * 2 * TW)
                    ov = out_sb[:, csl].rearrange("p (b w) -> p b w", w=TW)
                    nc.scalar.activation(
                        ov, opv, mybir.ActivationFunctionType.Copy,
                    )
                    if cth % 2 == 1:
                        hsl = slice((cth - 1) * 2 * TW, (cth + 1) * 2 * TW)
                        nc.sync.dma_start(out=out_ext[n, :, hsl],
                                          in_=out_sb[:, hsl])

                if n + 1 < N_PER:
                    xres = emit_xres(n + 1, nxt[0], nxt[1])

    nc.compile()
    return nc


def _prep_consts(bn1_gamma, bn1_beta, bn1_mean, bn1_var,
                 bn2_gamma, bn2_beta, bn2_mean, bn2_var, w1, w2, shift):
    s1 = bn1_gamma / np.sqrt(bn1_var + EPS)
    t1 = bn1_beta - bn1_mean * s1
    bias_full = (t1 / s1).astype(np.float32)  # [192] over concat channels
    bias1 = np.zeros((CP, 2), np.float32)
    bias1[0:96, 0] = bias_full[0:96]          # g0: x0 @ p0:48, f1 @ p48:96
    bias1[0:48, 1] = bias_full[144:192]       # g1: f0 @ p0:48
    bias1[64:112, 1] = bias_full[96:144]      # g1: x1 @ p64:112

    # padded index for original fmap channel c
    pidx = np.concatenate([np.arange(48), 64 + np.arange(48)])  # [96]

    s2f = bn2_gamma / np.sqrt(bn2_var + EPS)
    b2f = bn2_beta - bn2_mean * s2f
    t2 = np.zeros((CP, 1), np.float32)
    t2[pidx, 0] = b2f / s2f

    w1m = w1[:, :, 0, 0]  # (96 out, 96 in-per-group)
    w1t = np.zeros((CP, CP), np.float32)
    w1t[0:96, 0:48] = (w1m[0:48] * s1[None, 0:96]).T    # group0 lhsT [96K, 48M]
    # group1 SBUF rows: f0 @ p0:48, zeros @ p48:64, x1 @ p64:112
    w1g1 = (w1m[48:96] * s1[None, 96:192]).T            # row i = group1 input i
    w1t[0:48, 64:112] = w1g1[48:96]
    w1t[64:112, 64:112] = w1g1[0:48]

    dy, dx = shift[:, 0].astype(np.float64), shift[:, 1].astype(np.float64)
    ay = np.floor(dy)
    ax = np.floor(dx)
    fy = dy - ay
    fx = dx - ax
    wrf = np.zeros((C, 3), np.float32)
    wcf = np.zeros((C, 3), np.float32)
    for c in range(C):
        iy = int(ay[c]) + 1   # -1 -> 0, 0 -> 1
        ix = int(ax[c]) + 1
        wrf[c, iy] += 1.0 - fy[c]
        wrf[c, iy + 1] += fy[c]
        wcf[c, ix] += 1.0 - fx[c]
        wcf[c, ix + 1] += fx[c]
    wr = np.zeros((CP, 3), np.float32)
    wr[pidx] = wrf * s2f[:, None]

    w2m = w2[:, :, 0, 0]  # (96 out, 32 in-per-group)
    w2full = np.zeros((C, C), np.float32)
    for g in range(3):
        w2full[32 * g:32 * g + 32, 32 * g:32 * g + 32] = w2m[32 * g:32 * g + 32]
    w2x = np.zeros((CP, 288), np.float32)
    for k in range(3):
        # lhsT[pidx[c], o] = w2full[o, c] * wc[c, k]
        w2x[pidx, 96 * k:96 * k + 96] = w2full.T * wcf[:, k:k + 1]

    # residual: identity matmul from the 112-partition xres tile
    resw = np.zeros((CP, 96), np.float32)
    resw[np.arange(48), np.arange(48)] = 1.0
    resw[64 + np.arange(48), 48 + np.arange(48)] = 1.0

    return {
        "bias1": bias1,
        "t2": t2,
        "w1t": w1t.astype(ml_dtypes.bfloat16),
        "w2x": w2x.astype(ml_dtypes.bfloat16),
        "wr": wr,
        "resw": resw.astype(ml_dtypes.bfloat16),
        "zeros16": np.zeros((16, PIX), ml_dtypes.bfloat16),
    }


_NC_CACHE = {}


def kernel(x, prev_fmap, bn1_gamma, bn1_beta, bn1_mean, bn1_var,
           bn2_gamma, bn2_beta, bn2_mean, bn2_var, w1, w2, shift):
    global LAST_EXEC_NS
    x = np.ascontiguousarray(np.asarray(x, np.float32))
    prev_fmap = np.ascontiguousarray(np.asarray(prev_fmap, np.float32))
    consts = _prep_consts(
        np.asarray(bn1_gamma, np.float32), np.asarray(bn1_beta, np.float32),
        np.asarray(bn1_mean, np.float32), np.asarray(bn1_var, np.float32),
        np.asarray(bn2_gamma, np.float32), np.asarray(bn2_beta, np.float32),
        np.asarray(bn2_mean, np.float32), np.asarray(bn2_var, np.float32),
        np.asarray(w1, np.float32), np.asarray(w2, np.float32),
        np.asarray(shift, np.float32))

    if "nc" not in _NC_CACHE:
        _NC_CACHE["nc"] = _build_nc()
    nc = _NC_CACHE["nc"]

    NB = x.shape[0]
    xs = x.reshape(N_CORES, N_PER, C, PIX)
    ps = prev_fmap.reshape(N_CORES, N_PER, C, PIX)
    in_maps = [
        {"x": xs[i], "prev": ps[i], **consts}
        for i in range(N_CORES)
    ]

    trace = bool(os.environ.get("CC_KERNEL_TRACE"))
    res = run_bass_kernel_spmd(
        nc, in_maps, core_ids=list(range(N_CORES)), trace=trace,
    )
    LAST_EXEC_NS = res.exec_time_ns

    out = np.empty((NB, C, PIX), np.float32)
    fmap = np.empty((NB, C, PIX), np.float32)
    for i in range(N_CORES):
        out[i * N_PER:(i + 1) * N_PER] = res.results[i]["out"].astype(np.float32)
        fmap[i * N_PER:(i + 1) * N_PER] = res.results[i]["fmap"].astype(np.float32)
    return (out.reshape(NB, C, H, W), fmap.reshape(NB, C, H, W))



# revision 13
# speedup vs baseline: 1.5994x; 1.1174x over previous
"""Trainium2 Bass kernel for nn_BasicBlock (dense_cnn, active-shift block).

Data-parallel over batch: 32 images -> 4 per NeuronCore across 8 cores.
Per-core layout: channels on SBUF partitions, pixels (H*W) on the free dim.

Math restructure (validated vs the jax reference in fp32 to ~1e-7):
  - bn1+relu:  relu(s1*z + t1) = s1 * relu(z + t1/s1); the s1 scale is folded
    into the columns of w1, so bn1 is a single add+max tensor_scalar on
    VectorE (bf16, 4x mode).
  - conv1 (groups=2, bf16): two matmuls per pixel tile.  PE matmul outputs
    must start at partition 0 or 64, so the 96 fmap channels live interleaved
    on partitions [0:48] and [64:112]; partitions [48:64] are written zero via
    zero weight columns.  Everything after conv1 uses this padded
    112-partition layout (elementwise ops cost by free dim only, so the dead
    partitions are free); the fmap DMA and conv2 weights fold it back.
  - bn2+relu: ScalarE activation (per-partition scale/bias) from PSUM -> bf16.
  - active_shift is separable bilinear: a row pass on VectorE
    (v = wr0*b; bm = wrm*b; bp = wrp*b; v += shift(bm); v += shift(bp) --
    tensor_scalar 4x + tensor_tensor 2x only, no 1x-mode ops) and a column
    pass folded into conv2's weights (3 matmuls with column-shifted APs).
  - conv2 (groups=3) is a block-diagonal matmul over the padded layout; the
    +x residual is accumulated in PSUM via two shifted-identity matmuls from
    the bf16 raw tiles; ScalarE evicts the result.

dtype strategy: inputs are cast f32->bf16 by the load DMAs (GpSimd-initiated
casting DMAs; the GpSimd ALU pipeline stays empty -- its tensor ops are both
slow and poison concurrent VectorE ops via SBUF port sharing).  Outputs are
produced as bf16, DMA'd as bf16 (halves output HBM traffic) and widened to
f32 on the host.  End-to-end absmax-relative error ~3e-3.

Spatial tiling: 7 rows (392 px) per PSUM bank; pairs of banks share one PSUM
tile so bn2 / copies run at 784-px granularity (amortizes per-op overheads).
"""

import os
import numpy as np
import ml_dtypes

import concourse.bass as bass
import concourse.bacc as bacc
import concourse.mybir as mybir
from concourse import tile
from concourse.bass_utils import run_bass_kernel_spmd

EPS = 1e-5
N_CORES = 8
N_PER = 4            # images per core
C = 96
CP = 112             # padded channel count for the post-conv1 layout
H = 56
W = 56
PIX = H * W          # 3136
RT = 7               # rows per spatial tile
TW = RT * W          # 392 pixels per tile (one PSUM bank each)
NT = H // RT         # 8 tiles per image
NPAIR = NT // 2      # 4 two-bank chunks per image
BANK = 512           # fp32 elems per PSUM bank

f32 = mybir.dt.float32
bf16 = mybir.dt.bfloat16
f32r = mybir.dt.float32r

LAST_EXEC_NS = None


def _build_nc():
    nc = bacc.Bacc("TRN2", target_bir_lowering=False, debug=False, num_swdge_queues=4)

    x_ext = nc.declare_dram_parameter("x", [N_PER, C, PIX], f32, isOutput=False)
    p_ext = nc.declare_dram_parameter("prev", [N_PER, C, PIX], f32, isOutput=False)
    bias1_ext = nc.declare_dram_parameter("bias1", [C, 2], f32, isOutput=False)
    t2_ext = nc.declare_dram_parameter("t2", [CP, 1], f32, isOutput=False)
    w1t_ext = nc.declare_dram_parameter("w1t", [C, CP], bf16, isOutput=False)
    w2x_ext = nc.declare_dram_parameter("w2x", [CP, 288], bf16, isOutput=False)
    wr_ext = nc.declare_dram_parameter("wr", [CP, 3], f32, isOutput=False)
    resw_ext = nc.declare_dram_parameter("resw", [C, 96], bf16, isOutput=False)
    out_ext = nc.declare_dram_parameter("out", [N_PER, C, PIX], bf16, isOutput=True)
    fmap_ext = nc.declare_dram_parameter("fmap", [N_PER, C, PIX], bf16, isOutput=True)

    with tile.TileContext(nc) as tc:
        with (
            tc.tile_pool(name="consts", bufs=1) as cpool,
            tc.tile_pool(name="raw", bufs=2) as rawp,
            tc.tile_pool(name="act", bufs=2) as actp,
            tc.tile_pool(name="bv", bufs=2) as bvp,
            tc.tile_pool(name="outs", bufs=2) as outp,
            tc.tile_pool(name="fpsum", bufs=2, space="PSUM") as fpsum,
            tc.tile_pool(name="opsum", bufs=2, space="PSUM") as opsum,
        ):
            w1_sb = cpool.tile([C, CP], bf16)
            nc.sync.dma_start(out=w1_sb[:], in_=w1t_ext[:])
            w2_sb = cpool.tile([CP, 288], bf16)
            nc.sync.dma_start(out=w2_sb[:], in_=w2x_ext[:])
            wr_sb = cpool.tile([CP, 3], f32)
            nc.sync.dma_start(out=wr_sb[:], in_=wr_ext[:])
            bias1_sb = cpool.tile([C, 2], f32)
            nc.sync.dma_start(out=bias1_sb[:], in_=bias1_ext[:])
            t2_sb = cpool.tile([CP, 1], f32)
            nc.sync.dma_start(out=t2_sb[:], in_=t2_ext[:])
            resw_sb = cpool.tile([C, 96], bf16)
            nc.sync.dma_start(out=resw_sb[:], in_=resw_ext[:])

            HALF = PIX // 2

            def emit_loads(n):
                # group0 input = concat channels 0..95  = [x[0:48], prev[48:96]]
                # group1 input = concat channels 96..191 = [x[48:96], prev[0:48]]
                # casting DMAs (f32 -> bf16 in flight) via gpsimd rings.
                # image 0 loads at half-image granularity so bn1/conv1 can
                # start sooner (the gpsimd issue stream is the ramp limiter)
                g0_raw = rawp.tile([C, PIX], bf16, tag="g0raw", name=f"g0_raw{n}")
                g1_raw = rawp.tile([C, PIX], bf16, tag="g1raw", name=f"g1_raw{n}")
                parts = ((slice(0, HALF), slice(HALF, PIX)) if n == 0
                         else (slice(0, PIX),))
                for hs in parts:
                    nc.gpsimd.dma_start(out=g0_raw[0:48, hs], in_=x_ext[n, 0:48, hs])
                    nc.gpsimd.dma_start(out=g0_raw[48:96, hs], in_=p_ext[n, 48:96, hs])
                    nc.gpsimd.dma_start(out=g1_raw[0:48, hs], in_=x_ext[n, 48:96, hs])
                    nc.gpsimd.dma_start(out=g1_raw[48:96, hs], in_=p_ext[n, 0:48, hs])

                # bn1 + relu (scale folded into w1): a = max(z + bias1, 0)
                g0_act = actp.tile([C, PIX], bf16, tag="g0act", name=f"g0_act{n}")
                g1_act = actp.tile([C, PIX], bf16, tag="g1act", name=f"g1_act{n}")
                for hs in parts:
                    nc.vector.tensor_scalar(
                        g0_act[:, hs], g0_raw[:, hs], bias1_sb[:, 0:1], 0.0,
                        mybir.AluOpType.add, mybir.AluOpType.max,
                    )
                    nc.vector.tensor_scalar(
                        g1_act[:, hs], g1_raw[:, hs], bias1_sb[:, 1:2], 0.0,
                        mybir.AluOpType.add, mybir.AluOpType.max,
                    )
                return g0_raw, g1_raw, g0_act, g1_act

            def emit_xres(n, g0_raw, g1_raw):
                # contiguous bf16 copy of x for the single-matmul residual.
                # x's low half is partition-aligned with g0_raw, so VectorE
                # copies it (cheap 2x-mode op); the high half is misaligned
                # (g1_raw p0:48 -> xres p48:96) and goes via SBUF-SBUF DMA.
                xres = outp.tile([C, PIX], bf16, tag="xres", name=f"xres{n}")
                nc.vector.tensor_scalar(
                    xres[0:48, :], g0_raw[0:48, :], 1.0, None,
                    mybir.AluOpType.mult,
                )
                nc.sync.dma_start(out=xres[48:96, :], in_=g1_raw[0:48, :])
                return xres

            nxt = emit_loads(0)
            xres = emit_xres(0, nxt[0], nxt[1])
            for n in range(N_PER):
                g0_raw, g1_raw, g0_act, g1_act = nxt
                if n + 1 < N_PER:
                    nxt = emit_loads(n + 1)

                b_sb = bvp.tile([CP, PIX], bf16, tag="b")
                v_sb = bvp.tile([CP, PIX], bf16, tag="v")
                bm_sb = bvp.tile([CP, PIX], bf16, tag="bm")
                bp_sb = bvp.tile([CP, PIX], bf16, tag="bp")
                fmap_sb = outp.tile([CP, PIX], bf16, tag="fmap")
                out_sb = outp.tile([C, PIX], bf16, tag="out")

                # conv1 (groups=2) + bn2(relu) + fmap eviction, per 2-bank chunk
                for cth in range(NPAIR):
                    fp = fpsum.tile([CP, 2 * BANK], f32, tag="fp")
                    for k in range(2):
                        t = 2 * cth + k
                        sl = slice(t * TW, (t + 1) * TW)
                        pb = slice(k * BANK, k * BANK + TW)
                        nc.tensor.matmul(
                            fp[0:64, pb], w1_sb[:, 0:64],
                            g0_act[:, sl], start=True, stop=True,
                        )
                        nc.tensor.matmul(
                            fp[64:112, pb], w1_sb[:, 64:112],
                            g1_act[:, sl], start=True, stop=True,
                        )
                    fpv = fp.rearrange("p (b w) -> p b w", w=BANK)[:, :, 0:TW]
                    csl = slice(cth * 2 * TW, (cth + 1) * 2 * TW)
                    fv = fmap_sb[:, csl].rearrange("p (b w) -> p b w", w=TW)
                    nc.scalar.activation(
                        fv, fpv, mybir.ActivationFunctionType.Copy,
                    )
                    if cth % 2 == 1:
                        hsl = slice((cth - 1) * 2 * TW, (cth + 1) * 2 * TW)
                        nc.sync.dma_start(out=fmap_ext[n, 0:48, hsl],
                                          in_=fmap_sb[0:48, hsl])
                        nc.sync.dma_start(out=fmap_ext[n, 48:96, hsl],
                                          in_=fmap_sb[64:112, hsl])

                # row pass of the shift: v[c,i,:] = sum_oy wr[c,oy]*b[c,i+oy,:]
                # tensor_scalar (4x) + tensor_tensor (2x) only; no 1x STT ops.
                # Two halves, with the cross-half halo rows handled in the
                # second batch so every read refers to already-written data.
                HALF = PIX // 2
                for h0, h1 in ((0, HALF), (HALF, PIX)):
                    hs = slice(h0, h1)
                    # bn2 (scale folded into wr): b' = max(fmap + b2/s2, 0)
                    nc.vector.tensor_scalar(
                        b_sb[:, hs], fmap_sb[:, hs], t2_sb[:, 0:1], 0.0,
                        mybir.AluOpType.add, mybir.AluOpType.max,
                    )
                    nc.vector.tensor_scalar(
                        v_sb[:, hs], b_sb[:, hs], wr_sb[:, 1:2], None,
                        mybir.AluOpType.mult,
                    )
                    nc.vector.tensor_scalar(
                        bm_sb[:, hs], b_sb[:, hs], wr_sb[:, 0:1], None,
                        mybir.AluOpType.mult,
                    )
                    nc.vector.tensor_scalar(
                        bp_sb[:, hs], b_sb[:, hs], wr_sb[:, 2:3], None,
                        mybir.AluOpType.mult,
                    )
                    if h0 == 0:
                        # rows 1..27: bm rows 0..26 ; rows 0..26: bp rows 1..27
                        nc.vector.tensor_tensor(
                            v_sb[:, W:HALF], bm_sb[:, 0:HALF - W], v_sb[:, W:HALF],
                            mybir.AluOpType.add,
                        )
                        nc.vector.tensor_tensor(
                            v_sb[:, 0:HALF - W], bp_sb[:, W:HALF], v_sb[:, 0:HALF - W],
                            mybir.AluOpType.add,
                        )
                    else:
                        # rows 28..55: bm rows 27..54 ; rows 27..54: bp rows 28..55
                        nc.vector.tensor_tensor(
                            v_sb[:, HALF:PIX], bm_sb[:, HALF - W:PIX - W],
                            v_sb[:, HALF:PIX], mybir.AluOpType.add,
                        )
                        nc.vector.tensor_tensor(
                            v_sb[:, HALF - W:PIX - W], bp_sb[:, HALF:PIX],
                            v_sb[:, HALF - W:PIX - W], mybir.AluOpType.add,
                        )

                v3 = v_sb.rearrange("p (r w) -> p r w", w=W)

                # conv2 (col taps folded into weights) + residual, then evict
                for cth in range(NPAIR):
                    op = opsum.tile([C, 2 * BANK], f32, tag="op")
                    for k in range(2):
                        t = 2 * cth + k
                        sl = slice(t * TW, (t + 1) * TW)
                        pb = slice(k * BANK, k * BANK + TW)
                        r0 = t * RT
                        op3 = op[:, pb].rearrange("p (r w) -> p r w", w=W)
                        nc.tensor.matmul(
                            op[:, pb], w2_sb[:, 96:192], v_sb[:, sl],
                            start=True, stop=False, skip_group_check=True,
                        )
                        nc.tensor.matmul(
                            op3[:, :, 1:W], w2_sb[:, 0:96],
                            v3[:, r0:r0 + RT, 0:W - 1],
                            start=False, stop=False, skip_group_check=True,
                        )
                        nc.tensor.matmul(
                            op3[:, :, 0:W - 1], w2_sb[:, 192:288],
                            v3[:, r0:r0 + RT, 1:W],
                            start=False, stop=False, skip_group_check=True,
                        )
                        nc.tensor.matmul(
                            op[:, pb], resw_sb[:], xres[:, sl],
                            start=False, stop=True, skip_group_check=True,
                        )
                    opv = op.rearrange("p (b w) -> p b w", w=BANK)[:, :, 0:TW]
                    csl = slice(cth * 2 * TW, (cth + 1) * 2 * TW)
                    ov = out_sb[:, csl].rearrange("p (b w) -> p b w", w=TW)
                    nc.scalar.activation(
                        ov, opv, mybir.ActivationFunctionType.Copy,
                    )
                    if cth % 2 == 1:
                        hsl = slice((cth - 1) * 2 * TW, (cth + 1) * 2 * TW)
                        nc.sync.dma_start(out=out_ext[n, :, hsl],
                                          in_=out_sb[:, hsl])

                # xres for the next image, emitted last so the Vector queue
                # and sync DMA queue stay clear for this image's work
                if n + 1 < N_PER:
                    xres = emit_xres(n + 1, nxt[0], nxt[1])

    nc.compile()
    return nc


def _prep_consts(bn1_gamma, bn1_beta, bn1_mean, bn1_var,
                 bn2_gamma, bn2_beta, bn2_mean, bn2_var, w1, w2, shift):
    s1 = bn1_gamma / np.sqrt(bn1_var + EPS)
    t1 = bn1_beta - bn1_mean * s1
    bias1 = (t1 / s1).astype(np.float32).reshape(2, C).T.copy()  # [96, 2]

    # padded index for original fmap channel c
    pidx = np.concatenate([np.arange(48), 64 + np.arange(48)])  # [96]

    s2f = bn2_gamma / np.sqrt(bn2_var + EPS)
    b2f = bn2_beta - bn2_mean * s2f
    t2 = np.zeros((CP, 1), np.float32)
    t2[pidx, 0] = b2f / s2f

    w1m = w1[:, :, 0, 0]  # (96 out, 96 in-per-group)
    w1t = np.zeros((C, CP), np.float32)
    w1t[:, 0:48] = (w1m[0:48] * s1[None, 0:96]).T       # group0 lhsT [96K, 48M]
    w1t[:, 64:112] = (w1m[48:96] * s1[None, 96:192]).T  # group1 lhsT

    dy, dx = shift[:, 0].astype(np.float64), shift[:, 1].astype(np.float64)
    ay = np.floor(dy)
    ax = np.floor(dx)
    fy = dy - ay
    fx = dx - ax
    wrf = np.zeros((C, 3), np.float32)
    wcf = np.zeros((C, 3), np.float32)
    for c in range(C):
        iy = int(ay[c]) + 1   # -1 -> 0, 0 -> 1
        ix = int(ax[c]) + 1
        wrf[c, iy] += 1.0 - fy[c]
        wrf[c, iy + 1] += fy[c]
        wcf[c, ix] += 1.0 - fx[c]
        wcf[c, ix + 1] += fx[c]
    wr = np.zeros((CP, 3), np.float32)
    wr[pidx] = wrf * s2f[:, None]

    w2m = w2[:, :, 0, 0]  # (96 out, 32 in-per-group)
    w2full = np.zeros((C, C), np.float32)
    for g in range(3):
        w2full[32 * g:32 * g + 32, 32 * g:32 * g + 32] = w2m[32 * g:32 * g + 32]
    w2x = np.zeros((CP, 288), np.float32)
    for k in range(3):
        # lhsT[pidx[c], o] = w2full[o, c] * wc[c, k]
        w2x[pidx, 96 * k:96 * k + 96] = w2full.T * wcf[:, k:k + 1]

    # residual: identity matmul from the contiguous xres tile
    resw = np.eye(C, dtype=np.float32)

    return {
        "bias1": bias1,
        "t2": t2,
        "w1t": w1t.astype(ml_dtypes.bfloat16),
        "w2x": w2x.astype(ml_dtypes.bfloat16),
        "wr": wr,
        "resw": resw.astype(ml_dtypes.bfloat16),
    }


_NC_CACHE = {}


def kernel(x, prev_fmap, bn1_gamma, bn1_beta, bn1_mean, bn1_var,
           bn2_gamma, bn2_beta, bn2_mean, bn2_var, w1, w2, shift):
    global LAST_EXEC_NS
    x = np.ascontiguousarray(np.asarray(x, np.float32))
    prev_fmap = np.ascontiguousarray(np.asarray(prev_fmap, np.float32))
    consts = _prep_consts(
        np.asarray(bn1_gamma, np.float32), np.asarray(bn1_beta, np.float32),
        np.asarray(bn1_mean, np.float32), np.asarray(bn1_var, np.float32),
        np.asarray(bn2_gamma, np.float32), np.asarray(bn2_beta, np.float32),
        np.asarray(bn2_mean, np.float32), np.asarray(bn2_var, np.float32),
        np.asarray(w1, np.float32), np.asarray(w2, np.float32),
        np.asarray(shift, np.float32))

    if "nc" not in _NC_CACHE:
        _NC_CACHE["nc"] = _build_nc()
    nc = _NC_CACHE["nc"]

    NB = x.shape[0]
    xs = x.reshape(N_CORES, N_PER, C, PIX)
    ps = prev_fmap.reshape(N_CORES, N_PER, C, PIX)
    in_maps = [
        {"x": xs[i], "prev": ps[i], **consts}
        for i in range(N_CORES)
    ]

    trace = bool(os.environ.get("CC_KERNEL_TRACE"))
    res = run_bass_kernel_spmd(
        nc, in_maps, core_ids=list(range(N_CORES)), trace=trace,
    )
    LAST_EXEC_NS = res.exec_time_ns

    out = np.empty((NB, C, PIX), np.float32)
    fmap = np.empty((NB, C, PIX), np.float32)
    for i in range(N_CORES):
        out[i * N_PER:(i + 1) * N_PER] = res.results[i]["out"].astype(np.float32)
        fmap[i * N_PER:(i + 1) * N_PER] = res.results[i]["fmap"].astype(np.float32)
    return (out.reshape(NB, C, H, W), fmap.reshape(NB, C, H, W))

